# revision 4
# baseline (speedup 1.0000x reference)
"""Trainium2 Bass kernel for ContinuousNeuralField (gnn_message_passing).

Strategy (8 NeuronCores, SPMD):
  * Sort neurons by clipped x-coordinate on the host; core r owns the sorted
    band of N/8 = 1000 neurons (rows of the NxN connection matrix).
  * Connection weights are only nonzero for pairs with dist < max_r, and
    dist >= |dx|, so each band only interacts with a contiguous window of
    sorted neurons. Each core generates its [band x window] block of the
    (unnormalized) connection matrix on-chip and keeps it in SBUF (fp32).
  * d2 and feature-similarity are computed directly by TensorE via augmented
    K=5 / K=32 matmuls into PSUM; ACT does sqrt/exp, DVE does masking and
    products. Row-normalization is folded to after the message matmul using
    an extra ones-column (rowsum) on the activation operand.
  * Each message-passing iteration: matmul (cw block as stationary operand,
    activations as moving operand) -> per-window partial [8000+pad, 65] ->
    ReduceScatter(add) -> each core gets its own band's messages+rowsums ->
    local epilogue (divide, add, thresholds, relu, clamp).
  * Output: per-core partial (act*ow) @ OW -> AllReduce.

All per-core asymmetry (band/window slices) is carried in the per-core input
data; the single per-core data-dependent placement (where the window sits in
the ReduceScatter input) is a dynamic-offset DMA driven by a per-core scalar.
"""

import sys

sys.path.insert(0, "/opt/trn_rl_repo")

import numpy as np

# hardcoded problem geometry (from the problem spec)
M = 8          # cores
N = 8000       # neurons
B = 64         # batch
IN = 784       # input dim
OUT = 10       # output dim
FD = 32        # feature dim
VOL = 100.0
B0 = N // M    # band size (1000)
NJT = 8        # j-tiles per band (1024 padded)
BJ = NJT * 128  # padded band (1024)
KP = 896       # padded input dim (7*128)
NKT = KP // 128

_CACHE = {}
_LAST_RESULTS = None  # for test harness introspection


def _host_prep(x, positions, input_weights, features, output_weights,
               connection_radii, thresholds, n_iterations):
    f32 = np.float32
    x = np.asarray(x, f32)
    positions = np.asarray(positions, f32)
    input_weights = np.asarray(input_weights, f32)
    features = np.asarray(features, f32)
    output_weights = np.asarray(output_weights, f32)
    connection_radii = np.asarray(connection_radii, f32).reshape(N)
    thresholds = np.asarray(thresholds, f32).reshape(N)
    n_iters = int(np.asarray(n_iterations).reshape(-1)[0])

    pos = np.clip(positions, 0.1, VOL - 0.1)
    radii = np.clip(connection_radii, 1.0, 50.0)
    max_r = f32(radii.max())

    order = np.argsort(pos[:, 0], kind="stable")
    pos_s = pos[order]
    radii_s = radii[order]
    thr_s = thresholds[order]
    W_s = input_weights[order]
    F_s = features[order]
    OW_s = output_weights[order]

    xc = np.clip(pos_s[:, 0] / f32(VOL), 0.0, 1.0).astype(f32)
    iw = np.exp(f32(-3.0) * xc).astype(f32)
    iw = iw / (iw.sum(dtype=f32) + f32(1e-6))
    ow = np.exp(f32(3.0) * (xc - f32(1.0))).astype(f32)
    ow = ow / (ow.sum(dtype=f32) + f32(1e-6))

    nrm = np.sqrt((F_s.astype(f32) ** 2).sum(axis=1, dtype=f32)).astype(f32)
    fn = F_s / np.maximum(nrm, f32(1e-6))[:, None]

    wrec = (f32(1.0) / (radii_s + f32(1e-6))).astype(f32)   # 1/(r+eps)
    wi2 = (wrec * wrec).astype(f32)
    c2_all = (max_r * wrec).astype(f32) ** 2                # mask threshold on z2
    uniform = bool(np.all(radii_s == radii_s[0]))
    c2_imm = float(c2_all[0]) if uniform else None

    pc = (pos_s - f32(VOL / 2)).astype(f32)                 # centered positions
    n2 = (pc * pc).sum(axis=1, dtype=f32).astype(f32)

    # u_j . v_i = wi2_i * (|pj|^2 - 2 pj.pi + |pi|^2) = d2 * wi2_i
    U_all = np.stack([pc[:, 0], pc[:, 1], pc[:, 2], n2, np.ones(N, f32)]).astype(f32)
    V_all = np.stack([-2 * pc[:, 0] * wi2, -2 * pc[:, 1] * wi2,
                      -2 * pc[:, 2] * wi2, wi2, n2 * wi2]).astype(f32)
    FnT = np.ascontiguousarray(fn.T)                        # [32, N]

    xs = pos_s[:, 0]
    a_lo, Tn = [], []
    for r in range(M):
        lo, hi = r * B0, (r + 1) * B0
        w_lo = int(np.searchsorted(xs, xs[lo] - max_r, side="left"))
        w_hi = int(np.searchsorted(xs, xs[hi - 1] + max_r, side="right"))
        a_lo.append(w_lo)
        Tn.append(w_hi - w_lo)
    T = int(max(-(-t // 128) for t in Tn))                  # window tiles
    Twin = T * 128
    a_lo = [max(0, min(a, N - Twin)) for a in a_lo]

    Weff = (iw[:, None] * W_s).astype(f32)                  # fold input gating
    owOW = (ow[:, None] * OW_s).astype(f32)                 # fold output gating

    xtp = np.zeros((KP, B), f32)
    xtp[:IN] = x.T
    xt_k = np.ascontiguousarray(xtp.reshape(NKT, 128, B))

    per_core = []
    for r in range(M):
        lo, hi = r * B0, (r + 1) * B0
        al = a_lo[r]
        u5 = np.zeros((5, BJ), f32)
        u5[:, :B0] = U_all[:, lo:hi]
        u5[0, B0:] = 1e3      # far-away pad -> d2 huge -> cw exactly 0
        u5[3, B0:] = 1e6
        u5[4, B0:] = 1.0
        fnb = np.zeros((FD, BJ), f32)
        fnb[:, :B0] = FnT[:, lo:hi]
        wt = np.zeros((KP, BJ), f32)
        wt[:IN, :B0] = Weff[lo:hi].T
        weff_k = np.ascontiguousarray(wt.reshape(NKT, 128, BJ))
        thr_t = np.zeros((NJT, 128), f32)
        thr_t.reshape(-1)[:B0] = thr_s[lo:hi]
        oo = np.zeros((BJ, OUT), f32)
        oo[:B0] = owOW[lo:hi]
        oo_t = np.ascontiguousarray(oo.reshape(NJT, 128, OUT))
        onec = np.zeros((NJT, 128), f32)
        onec.reshape(-1)[:B0] = 1.0
        core = {
            "onec": onec,
            "u5": u5,
            "v5": np.ascontiguousarray(V_all[:, al:al + Twin]),
            "fnb": fnb,
            "fnw": np.ascontiguousarray(FnT[:, al:al + Twin]),
            "weff": weff_k,
            "xt": xt_k,
            "thr": thr_t,
            "owow": oo_t,
            "aoff": np.array([[al]], np.uint32),
        }
        if not uniform:
            core["c2w"] = np.ascontiguousarray(c2_all[al:al + Twin].reshape(1, Twin))
        per_core.append(core)

    return per_core, dict(T=T, n_iters=n_iters, uniform=uniform, c2_imm=c2_imm)


def _build(T, n_iters, uniform, c2_imm):
    import concourse.bass as bass
    import concourse.tile as tile
    from concourse import bacc, mybir
    from concourse.bass import _add_dep_helper

    f32 = mybir.dt.float32
    Twin = T * 128
    # chunking of the window free dim for generation
    chunks = []
    off = 0
    while off < Twin:
        cl = min(512, Twin - off)
        chunks.append((off, cl))
        off += cl

    nc = bacc.Bacc("TRN2", target_bir_lowering=False, debug=False, num_devices=M)
    u5_d = nc.dram_tensor("u5", [5, BJ], f32, kind="ExternalInput")
    v5_d = nc.dram_tensor("v5", [5, Twin], f32, kind="ExternalInput")
    fnb_d = nc.dram_tensor("fnb", [FD, BJ], f32, kind="ExternalInput")
    fnw_d = nc.dram_tensor("fnw", [FD, Twin], f32, kind="ExternalInput")
    weff_d = nc.dram_tensor("weff", [NKT, 128, BJ], f32, kind="ExternalInput")
    xt_d = nc.dram_tensor("xt", [NKT, 128, B], f32, kind="ExternalInput")
    thr_d = nc.dram_tensor("thr", [NJT, 128], f32, kind="ExternalInput")
    onec_d = nc.dram_tensor("onec", [NJT, 128], f32, kind="ExternalInput")
    owow_d = nc.dram_tensor("owow", [NJT, 128, OUT], f32, kind="ExternalInput")
    aoff_d = nc.dram_tensor("aoff", [1, 1], mybir.dt.uint32, kind="ExternalInput")
    if not uniform:
        c2w_d = nc.dram_tensor("c2w", [1, Twin], f32, kind="ExternalInput")
    out_d = nc.dram_tensor("out", [B, OUT], f32, kind="ExternalOutput")

    NPAD = N  # partial tensor rows
    with tile.TileContext(nc) as tc:
        with (
            tc.tile_pool(name="big", bufs=1) as big,
            tc.tile_pool(name="wchunk", bufs=3) as wch,
            tc.tile_pool(name="wk", bufs=10) as wk,
            tc.tile_pool(name="sm", bufs=8) as sm,
            tc.tile_pool(name="psg", bufs=4, space="PSUM") as psg,
            tc.tile_pool(name="psm", bufs=3, space="PSUM") as psm,
            tc.tile_pool(name="dram", bufs=1, space="DRAM") as dram,
        ):
            # ---- load static inputs
            u5 = big.tile([5, BJ], f32)
            nc.sync.dma_start(out=u5[:], in_=u5_d[:])
            v5 = big.tile([5, Twin], f32)
            nc.sync.dma_start(out=v5[:], in_=v5_d[:])
            fnb = big.tile([FD, BJ], f32)
            nc.sync.dma_start(out=fnb[:], in_=fnb_d[:])
            fnw = big.tile([FD, Twin], f32)
            nc.sync.dma_start(out=fnw[:], in_=fnw_d[:])
            xt = big.tile([128, NKT, B], f32)
            for kt in range(NKT):
                nc.sync.dma_start(out=xt[:, kt, :], in_=xt_d[kt])
            thr = big.tile([128, NJT], f32)
            for t in range(NJT):
                nc.sync.dma_start(out=thr[:, t:t + 1], in_=thr_d[t:t + 1, :])
            owow = big.tile([128, NJT, OUT], f32)
            for t in range(NJT):
                nc.sync.dma_start(out=owow[:, t, :], in_=owow_d[t])
            aoft = big.tile([1, 1], mybir.dt.uint32)
            nc.sync.dma_start(out=aoft[:], in_=aoff_d[:])
            if not uniform:
                c2b = big.tile([128, Twin], f32)
                nc.sync.dma_start(
                    out=c2b[:],
                    in_=bass.AP(tensor=c2w_d, offset=0,
                                ap=[[0, 128]] + c2w_d.ap().ap[1:]),
                )
            row_val = nc.gpsimd.value_load(aoft[0:1, 0:1])

            zero = big.tile([128, B + 1], f32)
            nc.vector.memset(zero[:], 0.0)
            b03 = big.tile([128, 1], f32)
            nc.vector.memset(b03[:], 0.3)

            actT = big.tile([128, NJT, B + 1], f32)
            cw = big.tile([128, NJT, Twin], f32)

            # ---- DRAM scratch
            partial_d = dram.tile([NPAD, B + 1], f32)
            rs_out_d = dram.tile([B0, B + 1], f32)
            ar_in_d = dram.tile([B, OUT], f32)
            ar_out_d = dram.tile([B, OUT], f32)

            # zero-fill the reduce-scatter input once
            nzt = -(-NPAD // 128)
            for t in range(nzt):
                rows = min(128, NPAD - t * 128)
                nc.sync.dma_start(out=partial_d[t * 128:t * 128 + rows, :],
                                  in_=zero[:rows, :])

            # ---- input projection: actT[:, it, 0:B] = (Weff @ x.T) band slice
            for it in range(NJT):
                pa = psm.tile([128, B], f32, tag="mm")
                for kt in range(NKT):
                    w = wch.tile([128, 128], f32, tag="w")
                    nc.sync.dma_start(out=w[:],
                                      in_=weff_d[kt, :, it * 128:(it + 1) * 128])
                    nc.tensor.matmul(pa[:], w[:], xt[:, kt, :],
                                     start=(kt == 0), stop=(kt == NKT - 1))
                nc.scalar.copy(actT[:, it, 0:B], pa[:])
            # ones column (zero on pad rows of the last band tile); pad-row
            # value cols are already 0 from the zero-padded Weff matmul.
            for t in range(NJT):
                nc.sync.dma_start(out=actT[:, t, B:B + 1], in_=onec_d[t:t + 1, :])

            # ---- generate cw block: [j-band x i-window], tiles [128, chunk]
            for jt in range(NJT):
                ujt = u5[:, jt * 128:(jt + 1) * 128]
                fjt = fnb[:, jt * 128:(jt + 1) * 128]
                for (co, cl) in chunks:
                    d2p = psg.tile([128, 512], f32, tag="gen")
                    nc.tensor.matmul(d2p[:, :cl], ujt, v5[:, co:co + cl],
                                     start=True, stop=True)
                    fsp = psg.tile([128, 512], f32, tag="gen")
                    nc.tensor.matmul(fsp[:, :cl], fjt, fnw[:, co:co + cl],
                                     start=True, stop=True)
                    z2 = wk.tile([128, 512], f32, tag="wk")
                    nc.vector.tensor_scalar(z2[:, :cl], d2p[:, :cl], 0.0, None,
                                            mybir.AluOpType.max)
                    m = wk.tile([128, 512], f32, tag="wk")
                    if uniform:
                        nc.vector.tensor_scalar(m[:, :cl], d2p[:, :cl], 0.0, c2_imm,
                                                mybir.AluOpType.max,
                                                mybir.AluOpType.is_lt)
                    else:
                        nc.vector.tensor_tensor(m[:, :cl], z2[:, :cl],
                                                c2b[:, co:co + cl],
                                                mybir.AluOpType.is_lt)
                    zr = wk.tile([128, 512], f32, tag="wk")
                    nc.scalar.activation(zr[:, :cl], z2[:, :cl],
                                         mybir.ActivationFunctionType.Sqrt)
                    e = wk.tile([128, 512], f32, tag="wk")
                    nc.scalar.activation(e[:, :cl], zr[:, :cl],
                                         mybir.ActivationFunctionType.Exp,
                                         bias=0.0, scale=-1.0)
                    f1 = wk.tile([128, 512], f32, tag="wk")
                    nc.scalar.activation(f1[:, :cl], fsp[:, :cl],
                                         mybir.ActivationFunctionType.Identity,
                                         bias=b03[:], scale=0.7)
                    q = wk.tile([128, 512], f32, tag="wk")
                    nc.vector.tensor_mul(q[:, :cl], e[:, :cl], m[:, :cl])
                    nc.vector.tensor_mul(cw[:, jt, co:co + cl], q[:, :cl],
                                         f1[:, :cl])

            # ---- message-passing iterations
            prev_rs = None
            for it_n in range(n_iters):
                dyn_dmas = []
                for itw in range(T):
                    pp = psm.tile([128, B + 1], f32, tag="mm")
                    for jt in range(NJT):
                        nc.tensor.matmul(pp[:],
                                         cw[:, jt, itw * 128:(itw + 1) * 128],
                                         actT[:, jt, :],
                                         start=(jt == 0), stop=(jt == NJT - 1))
                    stage = sm.tile([128, B + 1], f32, tag="sm")
                    nc.scalar.copy(stage[:], pp[:])
                    d = nc.gpsimd.dma_start(
                        out=partial_d[bass.ds(row_val + itw * 128, 128), :],
                        in_=stage[:],
                        bounds_check="skip_entire_dma",
                    )
                    if prev_rs is not None:
                        _add_dep_helper(d.ins, prev_rs.ins, True,
                                        "iter partial write after previous RS read")
                    dyn_dmas.append(d)
                cc = nc.gpsimd.collective_compute(
                    "ReduceScatter", mybir.AluOpType.add,
                    replica_groups=[list(range(M))],
                    ins=[partial_d[:]], outs=[rs_out_d[:]],
                )
                for d in dyn_dmas:
                    _add_dep_helper(cc.ins, d.ins, True,
                                    "RS reads dynamically-written partials")
                prev_rs = cc
                # epilogue: act = min(relu(act + msg/(rs+eps) - thr), 100)
                for t in range(NJT):
                    rows = min(128, B0 - t * 128)
                    if rows <= 0:
                        break
                    rsb = sm.tile([128, B + 1], f32, tag="sm")
                    nc.sync.dma_start(out=rsb[:rows, :],
                                      in_=rs_out_d[t * 128:t * 128 + rows, :])
                    rcp = sm.tile([128, 1], f32, tag="rc")
                    nc.vector.tensor_scalar(rcp[:rows], rsb[:rows, B:B + 1],
                                            1e-6, None, mybir.AluOpType.add)
                    nc.vector.reciprocal(rcp[:rows], rcp[:rows])
                    tmp = sm.tile([128, B], f32, tag="tm")
                    nc.vector.tensor_scalar(tmp[:rows], rsb[:rows, 0:B],
                                            rcp[:rows], None,
                                            mybir.AluOpType.mult)
                    nc.vector.tensor_add(tmp[:rows], tmp[:rows],
                                         actT[:rows, t, 0:B])
                    nc.vector.tensor_scalar(tmp[:rows], tmp[:rows],
                                            thr[:rows, t:t + 1], 0.0,
                                            mybir.AluOpType.subtract,
                                            mybir.AluOpType.max)
                    nc.vector.tensor_scalar(actT[:rows, t, 0:B], tmp[:rows],
                                            100.0, None, mybir.AluOpType.min)

            # ---- output: partial (act*ow) @ OW then AllReduce
            po = psm.tile([B, OUT], f32, tag="mm")
            for t in range(NJT):
                nc.tensor.matmul(po[:], actT[:, t, 0:B], owow[:, t, :],
                                 start=(t == 0), stop=(t == NJT - 1))
            ost = sm.tile([B, OUT], f32, tag="os")
            nc.scalar.copy(ost[:], po[:])
            nc.sync.dma_start(out=ar_in_d[:], in_=ost[:])
            nc.gpsimd.collective_compute(
                "AllReduce", mybir.AluOpType.add,
                replica_groups=[list(range(M))],
                ins=[ar_in_d[:]], outs=[ar_out_d[:]],
            )
            nc.sync.dma_start(out=out_d[:], in_=ar_out_d[:])

    nc.compile()
    return nc


def kernel(**inputs):
    global _LAST_RESULTS
    from concourse.bass_utils import run_bass_kernel_spmd

    per_core, geo = _host_prep(**inputs)
    key = (geo["T"], geo["n_iters"], geo["uniform"], geo["c2_imm"])
    if key not in _CACHE:
        _CACHE[key] = _build(*key)
    nc = _CACHE[key]
    res = run_bass_kernel_spmd(nc, per_core, core_ids=list(range(M)))
    _LAST_RESULTS = res
    return np.asarray(res.results[0]["out"], np.float32)
